# revision 2
# baseline (speedup 1.0000x reference)
"""Trainium2 Bass kernel v2 for nn_Mel_Decoder.

Data-parallel over batch: 128 -> 16 per NeuronCore (8 cores).

Design (per core, per block of SB=32 decoder steps, pairs j = b*sbk + il):
  - w1enc (fp16) and enc (fp16, t-major) are SBUF-resident for the whole
    kernel; no per-block reload.
  - attention tanh: DVE fp16 4x tensor_scalar adds (w1enc + s per pair)
    into big x16 tiles; single ACT Tanh per FDJ pairs (FD=FDJ*2*512) with
    fp8e3 output e8.
  - scores: per-pair stationary e8 chunks (fp8 -> 4x fast weight load),
    moving v8 [128,1]; psum [t,pair] transposed layout; exp (no max sub);
    denominator via ones-matmul over t partitions; 1/den folded into d_dot.
  - GRU gates use tanh instead of sigmoid (sigma(x) = (1+tanh(x/2))/2) so
    the whole kernel uses ONE ACT table set (exp_and_others: exp/tanh/relu).
  - GRU states kept fp32 with fp16 mirrors (matmul moving operands).
  - software pipeline: att-GRU chain of block k+1 and GRU1/2 chain of
    block k-1 are emitted interleaved into block k's attention phase.
"""

import os
from contextlib import ExitStack

import numpy as np

import concourse.bass as bass  # noqa: F401
import concourse.mybir as mybir
import concourse.tile as tile
from concourse import bacc
from concourse.bass_utils import run_bass_kernel_spmd
from concourse.masks import make_identity

FP32 = mybir.dt.float32
FP16 = mybir.dt.float16
FP8 = mybir.dt.float8e3
AF = mybir.ActivationFunctionType
ALU = mybir.AluOpType

P = 128
H = 256
H2 = 128
G3 = 768
MEL = 80
R = 5
TENC = 512
TDEC = 1000
BS = 128
NCORE = 8
BL = BS // NCORE
NSTEP = int(os.environ.get("MELDEC_STEPS", TDEC // R))  # 200
SB = 32
FDJ = 4           # pairs per tanh ACT instruction (FD = FDJ*2*512 = 4096)

NPAIR = NSTEP * BL


def _blocks():
    out = []
    s = 0
    while s < NSTEP:
        out.append((s, min(SB, NSTEP - s)))
        s += SB
    return out


def ts(i, n):
    return slice(i * n, (i + 1) * n)


class Builder:
    def __init__(self, nc, tc):
        self.nc = nc
        self.tc = tc

    # --------------------------------------------------------------- helpers
    def load_transposed(self, pool, ps_pool, w_ap, Mdim, Kdim, name):
        """fp32 sbuf tile [128, ceil(K/128), M]; dst[p,kc,m] = w[m, kc*128+p]."""
        nc = self.nc
        KC = (Kdim + P - 1) // P
        dst = pool.tile([P, KC, Mdim], FP32, tag=name)
        if Kdim % P != 0:
            nc.vector.memset(dst[:], 0.0)
        nrc = (Mdim + P - 1) // P
        for rc in range(nrc):
            rcnt = min(P, Mdim - rc * P)
            wrow = pool.tile([P, Kdim], FP32, name="wstg", tag="wstg",
                             padded_shape=[P, G3])
            nc.sync.dma_start(wrow[:rcnt, :], w_ap[rc * P : rc * P + rcnt, :])
            for kc in range(KC):
                kf = min(P, Kdim - kc * P)
                pst = ps_pool.tile([P, P], FP32, name="tr", tag="tr")
                nc.tensor.transpose(
                    pst[:kf, :rcnt], wrow[:rcnt, kc * P : kc * P + kf],
                    self.ident[:rcnt, :rcnt],
                )
                nc.vector.tensor_copy(dst[:kf, kc, rc * P : rc * P + rcnt],
                                      pst[:kf, :rcnt])
        return dst

    def lt16(self, stage_pool, pool, ps_pool, w_ap, Mdim, Kdim, name,
             dbl_n=False):
        """Transposed weight as fp16.  dbl_n: scale cols [2H,3H) by 2
        (pre-doubled n-gate input rows for the gate fusion)."""
        nc = self.nc
        t32 = self.load_transposed(stage_pool, ps_pool, w_ap, Mdim, Kdim,
                                   "w32stg")
        KC = (Kdim + P - 1) // P
        t16 = pool.tile([P, KC, Mdim], FP16, tag=name)
        if dbl_n:
            assert Mdim == G3
            nc.vector.tensor_copy(t16[:, :, 0 : 2 * H], t32[:, :, 0 : 2 * H])
            nc.vector.tensor_scalar_mul(t16[:, :, 2 * H : G3],
                                        t32[:, :, 2 * H : G3], 2.0)
        else:
            nc.vector.tensor_copy(t16[:], t32[:])
        return t16

    def load_vec(self, pool, v_ap, L, name):
        nc = self.nc
        t = pool.tile([P, L // P], FP32, tag=name)
        nc.sync.dma_start(t[:], v_ap.rearrange("(c p) -> p c", p=P))
        return t

    # ---------------------------------------------------------------- setup
    def setup(self, ins, psp):
        nc = self.nc
        cp = self.const

        self.ident = cp.tile([P, P], FP32, name="ident", tag="ident")
        make_identity(nc, self.ident[:])
        self.ident16 = cp.tile([P, P], FP16, name="ident16", tag="ident16")
        nc.vector.tensor_copy(self.ident16[:], self.ident[:])

        with self.tc.tile_pool(name="wsetup", bufs=1) as wsp:
            lt = lambda ap, M, K, nm, dbl=False: self.lt16(
                wsp, cp, psp, ap, M, K, nm, dbl)
            self.att_whhT = lt(ins["att_whh"], G3, H, "att_whhT")
            self.g1_whhT = lt(ins["g1_whh"], G3, H, "g1_whhT")
            self.g2_whhT = lt(ins["g2_whh"], G3, H, "g2_whhT")
            self.g1_wihT = lt(ins["g1_wih"], G3, H, "g1_wihT", dbl=True)
            self.g2_wihT = lt(ins["g2_wih"], G3, H, "g2_wihT", dbl=True)
            self.w2T = lt(ins["w2"], H, H, "w2T")
            self.projT = lt(ins["proj_w"], H, 2 * H, "projT")
            self.outwT = lt(ins["out_w"], MEL * R, H, "outwT")
            self.w1T = lt(ins["w1"], H, H, "w1T")

        self.b1T = self.load_vec(cp, ins["b1"], H, "b1T")
        self.b2T = self.load_vec(cp, ins["b2"], H, "b2T")
        self.proj_bT = self.load_vec(cp, ins["proj_b"], H, "proj_bT")
        self.pre_b1T = self.load_vec(cp, ins["pre_b1"], H, "pre_b1T")
        self.pre_b2T = self.load_vec(cp, ins["pre_b2"], H2, "pre_b2T")

        vf = cp.tile([P, 2], FP32, name="vf", tag="vf")
        nc.sync.dma_start(vf[:], ins["v_w"][0].rearrange("(c p) -> p c", p=P))
        # v scaled x16 into fp8e3's normal range (|v|~0.05 is subnormal
        # territory otherwise); the 1/16 is folded into the Exp scale.
        self.v8 = cp.tile([P, 2], FP8, name="v8", tag="v8")
        nc.vector.tensor_scalar_mul(self.v8[:], vf[:], 16.0)

        self.ones_col16 = cp.tile([P, 1], FP16, name="ones_col16",
                                  tag="ones_col16")
        nc.vector.memset(self.ones_col16[:], 1.0)
        self.ones_row16 = cp.tile([1, P], FP16, name="ones_row16",
                                  tag="ones_row16")
        nc.vector.memset(self.ones_row16[:], 1.0)
        ones_row = cp.tile([1, P], FP32, name="ones_row", tag="ones_row")
        nc.vector.memset(ones_row[:], 1.0)

        self.gi_sb0 = cp.tile([P, 6, BL * SB], FP16, tag="gi_sb0")

        ob_row = cp.tile([1, MEL * R], FP32, name="ob_row", tag="ob_row")
        nc.sync.dma_start(ob_row[:], ins["out_b"][None, :])
        ps_ob = psp.tile([P, MEL * R], FP32, name="mm", tag="mm")
        nc.tensor.matmul(ps_ob[:], ones_row[:], ob_row[:], start=True,
                         stop=True)
        self.outbB = cp.tile([P, MEL * R], FP32, name="outbB", tag="outbB")
        nc.scalar.copy(self.outbB[:], ps_ob[:])

    # -------------------------------------------- enc16 + w1enc16 (SBUF-resident)
    def enc_setup(self, ins):
        nc, tc = self.nc, self.tc
        enc = ins["enc_vec"]
        cp = self.const
        # enc16[p, b, tc4, h]  : t on partitions (for d_dot stationary)
        self.enc16 = cp.tile([P, BL, 4, H], FP16, tag="enc16")
        # w1enc16[p, c, b, t]  : h on partitions (for the bias-add input)
        self.w1enc16 = cp.tile([P, 2, BL, TENC], FP16, tag="w1enc16")
        with tc.tile_pool(name="encs", bufs=3) as ep, \
             tc.tile_pool(name="ps_enc", bufs=2, space="PSUM") as psp:
            for b in range(BL):
                st32 = ep.tile([P, 4, H], FP32, name="enc_stg", tag="enc_stg")
                nc.sync.dma_start(st32[:],
                                  enc[b].rearrange("(t p) h -> p t h", p=P))
                nc.vector.tensor_copy(self.enc16[:, b], st32[:])
                # transpose to [h, t] (fp16) and matmul w1T
                encT = ep.tile([P, 2, TENC], FP16, name="encT", tag="encT")
                for t4 in range(4):
                    for hc in range(2):
                        pst = psp.tile([P, P], FP16, name="tr16", tag="tr16")
                        nc.tensor.transpose(pst[:],
                                            self.enc16[:, b, t4, ts(hc, P)],
                                            self.ident16[:])
                        nc.vector.tensor_copy(encT[:, hc, ts(t4, P)], pst[:])
                for m in range(2):
                    ps = psp.tile([P, TENC], FP32, name="mm", tag="mm")
                    for k in range(2):
                        nc.tensor.matmul(ps[:], self.w1T[:, k, ts(m, P)],
                                         encT[:, k, :],
                                         start=(k == 0), stop=(k == 1))
                    nc.scalar.copy(self.w1enc16[:, m, b], ps[:])

    # ------------------------------------------------- pre-net + gi (DRAM, fp16)
    def prenet(self, ins, gi_d):
        nc, tc = self.nc, self.tc
        dec = ins["decoder_input"]
        with tc.tile_pool(name="pre2", bufs=2) as pp, \
             tc.tile_pool(name="pre1", bufs=1) as pp1, \
             tc.tile_pool(name="ps_pre", bufs=2, space="PSUM") as psp:
            prew1T = self.lt16(pp1, pp1, psp, ins["pre_w1"], H, MEL,
                               "prew1T")
            prew2T = self.lt16(pp1, pp1, psp, ins["pre_w2"], H2, H,
                               "prew2T")
            att_wihT = self.lt16(pp1, pp1, psp, ins["att_wih"], G3, H2,
                                 "att_wihT")

            xsrT = pp1.tile([P, NPAIR], FP16, name="xsrT", tag="xsrT")
            nc.vector.memset(xsrT[:], 0.0)
            for s0, sbk in _blocks():
                gb = P // sbk
                for t0 in range((BL * sbk) // P):
                    xt = pp.tile([P, MEL], FP32, name="xsr_nat", tag="xsr_nat")
                    src = dec[t0 * gb : (t0 + 1) * gb,
                              s0 * R : (s0 + sbk) * R : R, :]
                    nc.sync.dma_start(xt[:], src)
                    pst = psp.tile([P, P], FP32, name="tr", tag="tr")
                    nc.tensor.transpose(pst[:MEL, :], xt[:, :], self.ident[:])
                    nc.scalar.copy(
                        xsrT[:MEL, BL * s0 + t0 * P : BL * s0 + (t0 + 1) * P],
                        pst[:MEL, :])

            # chunk-outer so block 0's gi lands early (A(0) overlaps the rest)
            for n0 in range(0, NPAIR, 512):
                nsz = min(512, NPAIR - n0)
                pre1T = pp.tile([P, 2, 512], FP16, name="pre1T", tag="pre1T")
                xsT = pp.tile([P, 512], FP16, name="xsT", tag="xsT")
                for m in range(2):
                    ps = psp.tile([P, 512], FP32, name="mm", tag="mm")
                    nc.tensor.matmul(ps[:, :nsz], prew1T[:, 0, ts(m, P)],
                                     xsrT[:, n0 : n0 + nsz],
                                     start=True, stop=True)
                    nc.vector.tensor_scalar(pre1T[:, m, :nsz], ps[:, :nsz],
                                            self.pre_b1T[:, m : m + 1], 0.0,
                                            ALU.add, ALU.max)
                ps = psp.tile([P, 512], FP32, name="mm", tag="mm")
                for k in range(2):
                    nc.tensor.matmul(ps[:, :nsz], prew2T[:, k, :],
                                     pre1T[:, k, :nsz],
                                     start=(k == 0), stop=(k == 1))
                nc.vector.tensor_scalar(xsT[:, :nsz], ps[:, :nsz],
                                        self.pre_b2T[:, 0:1], 0.0,
                                        ALU.add, ALU.max)
                for m in range(6):
                    ps = psp.tile([P, 512], FP32, name="mm", tag="mm")
                    nc.tensor.matmul(ps[:, :nsz], att_wihT[:, 0, ts(m, P)],
                                     xsT[:, :nsz], start=True, stop=True)
                    # block 0's gi goes straight to SBUF (no DRAM roundtrip,
                    # whose whole-tensor dep would stall A(0) on all of prenet)
                    dst = (self.gi_sb0[:, m, :nsz] if n0 == 0
                           else pp.tile([P, 512], FP16, name="gi_stage",
                                        tag="gi_stage")[:, :nsz])
                    if m >= 4:
                        # n-gate input pre-doubled for the gate fusion
                        nc.vector.tensor_scalar(dst, ps[:, :nsz],
                                                0.0, 2.0, ALU.add, ALU.mult)
                    else:
                        nc.vector.tensor_copy(dst, ps[:, :nsz])
                    if n0 != 0:
                        nc.sync.dma_start(gi_d[m, :, n0 : n0 + nsz], dst)

    # ---------------------------------------------------------------- gates
    def gates(self, rz_ps, hn_ps, gin, prev16, prev32, out32, out16):
        """GRU cell tail.  rz_ps: psum [128,4,BL] pre-activations for (r,z)
        (both H-chunks); hn_ps: psum [128,2,BL] recurrent n term; gin:
        PRE-DOUBLED n input term (sbuf fp16 or psum fp32).
        sigma(x) = (1+tanh(x/2))/2 so only Tanh is used.
        h' = 0.5*(h + n + tz*(h-n)), n = tanh(0.5*(2*gin + hn + tr*hn))."""
        nc = self.nc
        g = self.g_pool
        trz = g.tile([P, 4, BL], FP16, name="trz", tag="trz")
        nc.scalar.activation(trz[:], rz_ps, AF.Tanh, scale=0.5)
        s1 = g.tile([P, 2, BL], FP32, name="s1", tag="s1")
        nc.vector.tensor_mul(s1[:], trz[:, 0:2], hn_ps)
        s2 = g.tile([P, 2, BL], FP32, name="s2", tag="s2")
        nc.vector.tensor_add(s2[:], s1[:], hn_ps)
        s3 = g.tile([P, 2, BL], FP32, name="s3", tag="s3")
        nc.vector.tensor_add(s3[:], s2[:], gin)
        tn = g.tile([P, 2, BL], FP16, name="tn", tag="tn")
        nc.scalar.activation(tn[:], s3[:], AF.Tanh, scale=0.5)
        a = g.tile([P, 2, BL], FP16, name="ga", tag="ga")
        nc.vector.tensor_sub(a[:], prev16, tn[:])
        bq = g.tile([P, 2, BL], FP16, name="gb", tag="gb")
        nc.vector.tensor_mul(bq[:], trz[:, 2:4], a[:])
        q = g.tile([P, 2, BL], FP16, name="gq", tag="gq")
        nc.vector.tensor_add(q[:], bq[:], tn[:])
        t32 = g.tile([P, 2, BL], FP32, name="gt", tag="gt")
        nc.vector.tensor_add(t32[:], prev32, q[:])
        nc.vector.tensor_scalar_mul(out32, t32[:], 0.5)
        nc.vector.tensor_scalar_mul(out16, t32[:], 0.5)

    # ------------------------------------------------------------ chain steps
    def emit_A_step(self, il, sbk, bp, gi_blk, dT16, sT16):
        nc = self.nc
        sl = slice(il, bp, sbk)
        ps = self.ps_gruA.tile([P, 8, BL], FP32, name="gruA", tag="gruA")
        for m in range(4):
            nc.tensor.matmul(ps[:, m], self.att_whhT[:, 0, ts(m, P)],
                             self.d16_prev, start=True, stop=False)
            nc.tensor.matmul(ps[:, m], self.att_whhT[:, 1, ts(m, P)],
                             self.d16_prev2, start=False, stop=False)
            nc.tensor.matmul(ps[:, m], self.ident16[:], gi_blk[:, m, sl],
                             start=False, stop=True)
        for m in (4, 5):
            nc.tensor.matmul(ps[:, m], self.att_whhT[:, 0, ts(m, P)],
                             self.d16_prev, start=True, stop=False)
            nc.tensor.matmul(ps[:, m], self.att_whhT[:, 1, ts(m, P)],
                             self.d16_prev2, start=False, stop=True)
        d32 = self.st_pool.tile([P, 2, BL], FP32, name="dA32", tag="dA32")
        self.gates(ps[:, 0:4], ps[:, 4:6], gi_blk[:, 4:6, sl],
                   self.dfull16_prev, self.dfull32_prev,
                   d32[:], dT16[:, :, sl])
        self.d16_prev = dT16[:, 0, sl]
        self.d16_prev2 = dT16[:, 1, sl]
        self.dfull16_prev = dT16[:, :, sl]
        self.dfull32_prev = d32[:]
        # per-step sT = w2 @ d + b2 (lets the next block's attention start
        # before this chain is fully done)
        for m in range(2):
            for k in range(2):
                nc.tensor.matmul(ps[:, 6 + m], self.w2T[:, k, ts(m, P)],
                                 dT16[:, k, sl], start=(k == 0), stop=(k == 1))
            nc.vector.tensor_scalar_add(sT16[:, m, sl], ps[:, 6 + m],
                                        self.b2T[:, m : m + 1])

    def emit_G_step(self, il, sbk, bp, pT16, G1P16, sum2T16):
        nc = self.nc
        sl = slice(il, bp, sbk)
        # GRU1
        ps1 = self.ps_gru1.tile([P, 6, BL], FP32, name="gru1", tag="gru1")
        for m in range(4):
            nc.tensor.matmul(ps1[:, m], self.g1_whhT[:, 0, ts(m, P)],
                             self.o1_16[:, 0], start=True, stop=False)
            nc.tensor.matmul(ps1[:, m], self.g1_whhT[:, 1, ts(m, P)],
                             self.o1_16[:, 1], start=False, stop=False)
            nc.tensor.matmul(ps1[:, m], self.ident16[:], G1P16[:, m, sl],
                             start=False, stop=True)
        for m in (4, 5):
            nc.tensor.matmul(ps1[:, m], self.g1_whhT[:, 0, ts(m, P)],
                             self.o1_16[:, 0], start=True, stop=False)
            nc.tensor.matmul(ps1[:, m], self.g1_whhT[:, 1, ts(m, P)],
                             self.o1_16[:, 1], start=False, stop=True)
        st = self.st_pool
        o1n32 = st.tile([P, 2, BL], FP32, name="o1n32", tag="o1n32")
        o1n16 = st.tile([P, 2, BL], FP16, name="o1n16", tag="o1n16")
        self.gates(ps1[:, 0:4], ps1[:, 4:6], G1P16[:, 4:6, sl],
                   self.o1_16[:], self.o1_32[:], o1n32[:], o1n16[:])
        in2_32 = st.tile([P, 2, BL], FP32, name="in2_32", tag="in2_32")
        in2_16 = st.tile([P, 2, BL], FP16, name="in2_16", tag="in2_16")
        nc.vector.tensor_add(in2_32[:], o1n32[:], pT16[:, :, sl])
        nc.vector.tensor_copy(in2_16[:], in2_32[:])
        # GRU2: rz 0:4 (whh+wih), hn 4:6 (whh n), inn 6:8 (wih n, pre-doubled)
        ps2 = self.ps_gru2.tile([P, 8, BL], FP32, name="gru2", tag="gru2")
        for m in range(4):
            nc.tensor.matmul(ps2[:, m], self.g2_whhT[:, 0, ts(m, P)],
                             self.o2_16[:, 0], start=True, stop=False)
            nc.tensor.matmul(ps2[:, m], self.g2_whhT[:, 1, ts(m, P)],
                             self.o2_16[:, 1], start=False, stop=False)
            nc.tensor.matmul(ps2[:, m], self.g2_wihT[:, 0, ts(m, P)],
                             in2_16[:, 0], start=False, stop=False)
            nc.tensor.matmul(ps2[:, m], self.g2_wihT[:, 1, ts(m, P)],
                             in2_16[:, 1], start=False, stop=True)
        for m in range(2):
            nc.tensor.matmul(ps2[:, 4 + m], self.g2_whhT[:, 0, ts(4 + m, P)],
                             self.o2_16[:, 0], start=True, stop=False)
            nc.tensor.matmul(ps2[:, 4 + m], self.g2_whhT[:, 1, ts(4 + m, P)],
                             self.o2_16[:, 1], start=False, stop=True)
            nc.tensor.matmul(ps2[:, 6 + m], self.g2_wihT[:, 0, ts(4 + m, P)],
                             in2_16[:, 0], start=True, stop=False)
            nc.tensor.matmul(ps2[:, 6 + m], self.g2_wihT[:, 1, ts(4 + m, P)],
                             in2_16[:, 1], start=False, stop=True)
        o2n32 = st.tile([P, 2, BL], FP32, name="o2n32", tag="o2n32")
        o2n16 = st.tile([P, 2, BL], FP16, name="o2n16", tag="o2n16")
        self.gates(ps2[:, 0:4], ps2[:, 4:6], ps2[:, 6:8],
                   self.o2_16[:], self.o2_32[:], o2n32[:], o2n16[:])
        nc.vector.tensor_add(sum2T16[:, :, sl], in2_16[:], o2n16[:])
        self.o1_32, self.o1_16 = o1n32, o1n16
        self.o2_32, self.o2_16 = o2n32, o2n16

    # ---------------------------------------------------------------- phases
    def emit_T(self, sbk, bp, sT16, a_tokens, g_tokens, il_major=False):
        nc = self.nc
        ng = bp // P
        gb = P // sbk
        nchunk = bp // FDJ
        # A tokens (and the trailing S) run at every chunk from the start so
        # the next block's sT is ready early; G tokens spread over the rest.
        na = len(a_tokens)
        g_start = na + 1
        g_every = max(1, (nchunk - g_start) // max(1, len(g_tokens)))
        tokens = {}
        for i, t in enumerate(a_tokens):
            tokens[i] = t
        for i, t in enumerate(g_tokens):
            tokens[g_start + i * g_every] = t
        ti = 0
        expT = self.expT_pool.tile([P, 4, BL * SB], FP16, name="expT",
                                   tag="expT")[:, :, :bp]
        rdenB = self.expT_pool.tile([P, BL * SB], FP16, name="rdenB",
                                    tag="rdenB")[:, :bp]
        ps_dd = self.ps_dd.tile([P, 2, 512], FP32, name="dd",
                                tag="dd")[:, :, :bp]
        ps_s = None
        for g in range(ng):
            if il_major:
                starts = [b * sbk + il4 * FDJ
                          for il4 in range(sbk // FDJ)
                          for b in range(g * gb, (g + 1) * gb)]
            else:
                starts = [g * P + q * FDJ for q in range(P // FDJ)]
            for jq, j0 in enumerate(starts):
                x16 = self.x_pool.tile([P, FDJ, 2, TENC], FP16, name="x16",
                                       tag="x16")
                for jj in range(FDJ):
                    j = j0 + jj
                    b = j // sbk
                    for c in range(2):
                        nc.vector.tensor_scalar_add(
                            x16[:, jj, c], self.w1enc16[:, c, b],
                            sT16[:, c, j : j + 1])
                e8 = self.e_pool.tile([P, FDJ, 2, TENC], FP8, name="e8",
                                      tag="e8")
                nc.scalar.activation(e8[:], x16[:], AF.Tanh)
                if jq == 0:
                    ps_s = self.ps_sc.tile([P, 4, P], FP32, name="sc",
                                           tag="sc")
                for jj in range(FDJ):
                    j = j0 + jj
                    row = j % P
                    for t4 in range(4):
                        for c in range(2):
                            nc.tensor.matmul(
                                ps_s[:, t4, row : row + 1],
                                e8[:, jj, c, ts(t4, P)],
                                self.v8[:, c : c + 1],
                                start=(c == 0), stop=(c == 1))
                tok = tokens.pop(g * (P // FDJ) + jq, None)
                if tok is not None:
                    tok()
            gsl = ts(g, P)
            nc.scalar.activation(expT[:, :, gsl], ps_s[:], AF.Exp,
                                 scale=1.0 / 16.0)
            ps_den = self.ps_mm.tile([1, P], FP32, name="den", tag="mm")
            for t4 in range(4):
                nc.tensor.matmul(ps_den[0:1, :], self.ones_col16[:],
                                 expT[:, t4, gsl],
                                 start=(t4 == 0), stop=(t4 == 3))
            rden16 = self.g_pool.tile([1, P], FP16, name="rden", tag="rden")
            with nc.allow_low_precision(reason="softmax 1/den in fp16"):
                nc.vector.reciprocal(rden16[0:1, :], ps_den[0:1, :])
            ps_rb = self.ps_mm.tile([P, P], FP32, name="rb", tag="mm")
            nc.tensor.matmul(ps_rb[:], self.ones_row16[0:1, :],
                             rden16[0:1, :], start=True, stop=True)
            nc.vector.tensor_copy(rdenB[:, gsl], ps_rb[:])
            for bi in range(gb):
                b = g * gb + bi
                bs_ = slice(b * sbk, (b + 1) * sbk)
                for hc in range(2):
                    for k4 in range(4):
                        nc.tensor.matmul(ps_dd[:, hc, bs_],
                                         self.enc16[:, b, k4, ts(hc, P)],
                                         expT[:, k4, bs_],
                                         start=(k4 == 0), stop=(k4 == 3))
        for idx in sorted(tokens):
            tokens[idx]()
        tokens.clear()
        ddT16 = self.ddT_pool.tile([P, 2, BL * SB], FP16, name="ddT",
                                   tag="ddT")[:, :, :bp]
        for hc in range(2):
            nc.vector.tensor_mul(ddT16[:, hc], ps_dd[:, hc], rdenB[:])
        return ddT16

    def emit_P5(self, dT16, ddT16, bp):
        nc = self.nc
        pT16 = self.pT_pool.tile([P, 2, BL * SB], FP16, name="pT",
                                 tag="pT")[:, :, :bp]
        for m in range(2):
            ps = self.ps_mm.tile([P, 512], FP32, name="mm", tag="mm")[:, :bp]
            for k in range(4):
                rhs = dT16[:, k, :] if k < 2 else ddT16[:, k - 2, :]
                nc.tensor.matmul(ps[:], self.projT[:, k, ts(m, P)], rhs,
                                 start=(k == 0), stop=(k == 3))
            nc.vector.tensor_scalar_add(pT16[:, m], ps[:],
                                        self.proj_bT[:, m : m + 1])
        G1P16 = self.G1P_pool.tile([P, 6, BL * SB], FP16, name="G1P",
                                   tag="G1P")[:, :, :bp]
        for m in range(6):
            ps = self.ps_mm.tile([P, 512], FP32, name="mm", tag="mm")[:, :bp]
            for k in range(2):
                nc.tensor.matmul(ps[:], self.g1_wihT[:, k, ts(m, P)],
                                 pT16[:, k, :], start=(k == 0), stop=(k == 1))
            nc.vector.tensor_copy(G1P16[:, m], ps[:])
        return pT16, G1P16

    def emit_O(self, sum2T16, s0, sbk, bp, y):
        nc = self.nc
        ng = bp // P
        gb = P // sbk
        for t0 in range(ng):
            ps = self.ps_mm.tile([P, MEL * R], FP32, name="mm", tag="mm")
            for k in range(2):
                nc.tensor.matmul(ps[:], sum2T16[:, k, ts(t0, P)],
                                 self.outwT[:, k, :],
                                 start=(k == 0), stop=(k == 1))
            ot = self.o_pool.tile([P, MEL * R], FP32, name="out_sb",
                                  tag="out_sb")
            nc.vector.tensor_add(ot[:], ps[:], self.outbB[:])
            for bi in range(gb):
                b = t0 * gb + bi
                nc.sync.dma_start(
                    y[b, s0 * R : (s0 + sbk) * R, :].rearrange(
                        "(i r) m -> i (r m)", r=R),
                    ot[bi * sbk : (bi + 1) * sbk, :])

    # ------------------------------------------------------------------ main
    def main(self, ins, gi_d, y, stack):
        nc, tc = self.nc, self.tc
        ec = stack.enter_context
        self.st_pool = ec(tc.tile_pool(name="states", bufs=2))
        self.g_pool = ec(tc.tile_pool(name="gates", bufs=3))
        self.dT_pool = ec(tc.tile_pool(name="dT", bufs=2))
        self.sT_pool = ec(tc.tile_pool(name="sT", bufs=2))
        self.pT_pool = ec(tc.tile_pool(name="pT", bufs=2))
        self.G1P_pool = ec(tc.tile_pool(name="G1P", bufs=2))
        self.sum2_pool = ec(tc.tile_pool(name="sum2", bufs=2))
        self.ddT_pool = ec(tc.tile_pool(name="ddT", bufs=2))
        self.expT_pool = ec(tc.tile_pool(name="expT", bufs=2))
        self.x_pool = ec(tc.tile_pool(name="xbuf", bufs=2))
        self.e_pool = ec(tc.tile_pool(name="ebuf", bufs=2))
        self.gi_pool = ec(tc.tile_pool(name="gi", bufs=2))
        self.o_pool = ec(tc.tile_pool(name="obuf", bufs=2))
        self.ps_sc = ec(tc.tile_pool(name="ps_sc", bufs=1, space="PSUM"))
        self.ps_dd = ec(tc.tile_pool(name="ps_dd", bufs=1, space="PSUM"))
        self.ps_mm = ec(tc.tile_pool(name="ps_mm", bufs=2, space="PSUM"))
        self.ps_gruA = ec(tc.tile_pool(name="ps_gruA", bufs=1, space="PSUM"))
        self.ps_gru1 = ec(tc.tile_pool(name="ps_gru1", bufs=1, space="PSUM"))
        self.ps_gru2 = ec(tc.tile_pool(name="ps_gru2", bufs=1, space="PSUM"))

        zeros16 = self.st_pool.tile([P, 2, BL], FP16, name="z16", tag="z16")
        nc.vector.memset(zeros16[:], 0.0)
        zeros32 = self.st_pool.tile([P, 2, BL], FP32, name="z32", tag="z32")
        nc.vector.memset(zeros32[:], 0.0)
        self.d16_prev = zeros16[:, 0]
        self.d16_prev2 = zeros16[:, 1]
        self.dfull16_prev = zeros16[:]
        self.dfull32_prev = zeros32[:]
        self.o1_16 = zeros16
        self.o1_32 = zeros32
        self.o2_16 = zeros16
        self.o2_32 = zeros32

        blocks = _blocks()

        def load_gi(k):
            s0, sbk = blocks[k]
            bp = BL * sbk
            p0 = BL * s0
            gi_blk = self.gi_pool.tile([P, 6, BL * SB], FP16, name="gi_blk",
                                       tag="gi_blk")[:, :, :bp]
            nc.sync.dma_start(
                gi_blk[:],
                gi_d[:, :, p0 : p0 + bp].rearrange("c p j -> p c j"))
            return gi_blk

        def new_dT(bp_):
            dT = self.dT_pool.tile([P, 2, BL * SB], FP16, name="dT16",
                                   tag="dT16")[:, :, :bp_]
            sT = self.sT_pool.tile([P, 2, BL * SB], FP32, name="sT",
                                   tag="sT")[:, :, :bp_]
            return dT, sT

        # prologue: A(0) chain (computes sT(0) per step)
        gi_blk = self.gi_sb0
        s0, sbk = blocks[0]
        bp = BL * sbk
        dT16, sT16 = new_dT(bp)
        for il in range(sbk):
            self.emit_A_step(il, sbk, bp, gi_blk, dT16, sT16)

        dT16_cur = dT16
        sum2_prev = None
        prev_blk = None       # (s0, sbk, bp) of block k-1
        pT_prev = G1P_prev = None

        for k in range(len(blocks)):
            s0, sbk = blocks[k]
            bp = BL * sbk
            a_tokens = []
            g_tokens = []
            # A(k+1) tokens (+ S(k+1) right after the chain completes)
            if k + 1 < len(blocks):
                s0n, sbkn = blocks[k + 1]
                bpn = BL * sbkn
                gi_next = load_gi(k + 1)
                dT16n, sT16n = new_dT(bpn)
                for il in range(sbkn):
                    a_tokens.append(lambda il=il: self.emit_A_step(
                        il, sbkn, bpn, gi_next, dT16n, sT16n))
            # G(k-1) tokens
            if prev_blk is not None:
                ps0, psbk, pbp = prev_blk
                sum2_prev = self.sum2_pool.tile(
                    [P, 2, BL * SB], FP16, name="sum2", tag="sum2")[:, :, :pbp]
                for il in range(psbk):
                    g_tokens.append(lambda il=il, s=sum2_prev: self.emit_G_step(
                        il, psbk, pbp, pT_prev, G1P_prev, s))

            ddT16 = self.emit_T(sbk, bp, sT16, a_tokens, g_tokens,
                                il_major=(k == 0))
            if k + 1 < len(blocks):
                sT16 = sT16n
            pT_prev, G1P_prev = self.emit_P5(dT16_cur, ddT16, bp)
            if prev_blk is not None:
                self.emit_O(sum2_prev, prev_blk[0], prev_blk[1], prev_blk[2], y)
            prev_blk = (s0, sbk, bp)
            if k + 1 < len(blocks):
                dT16_cur = dT16n

        # epilogue: G(last) + O(last)
        ps0, psbk, pbp = prev_blk
        sum2_last = self.sum2_pool.tile([P, 2, BL * SB], FP16, name="sum2",
                                        tag="sum2")[:, :, :pbp]
        for il in range(psbk):
            self.emit_G_step(il, psbk, pbp, pT_prev, G1P_prev, sum2_last)
        self.emit_O(sum2_last, ps0, psbk, pbp, y)


def build(ins_np):
    for nm in ("att_bih", "att_bhh", "g1_bih", "g1_bhh", "g2_bih", "g2_bhh",
               "b1"):
        assert not np.asarray(ins_np[nm]).any(), f"nonzero gate bias {nm}"
    nc = bacc.Bacc()
    ins = {}
    for name, arr in ins_np.items():
        shp = list(np.asarray(arr).shape)
        if name in ("enc_vec", "decoder_input"):
            shp[0] = BL
        ins[name] = nc.declare_dram_parameter(name, shp, FP32, isOutput=False)
    y = nc.declare_dram_parameter("y", [BL, TDEC, MEL], FP32, isOutput=True)
    gi_d = nc.dram_tensor("gi_d", [6, P, NPAIR], FP16)

    with tile.TileContext(nc) as tc:
        with ExitStack() as stack:
            b = Builder(nc, tc)
            b.const = stack.enter_context(tc.tile_pool(name="const", bufs=1))
            with tc.tile_pool(name="ps_setup", bufs=2, space="PSUM") as psp:
                b.setup(ins, psp)
            b.prenet(ins, gi_d)
            b.enc_setup(ins)
            b.main(ins, gi_d, y, stack)
    nc.compile()
    return nc


_CACHE = {}


def kernel(**inputs):
    if "nc" not in _CACHE:
        _CACHE["nc"] = build(inputs)
    nc = _CACHE["nc"]
    in_maps = []
    for c in range(NCORE):
        m = {}
        for name, arr in inputs.items():
            a = np.asarray(arr, dtype=np.float32)
            if name in ("enc_vec", "decoder_input"):
                a = a[c * BL : (c + 1) * BL]
            m[name] = np.ascontiguousarray(a)
        in_maps.append(m)
    res = run_bass_kernel_spmd(nc, in_maps, list(range(NCORE)))
    return np.concatenate([res.results[c]["y"] for c in range(NCORE)], axis=0)


# revision 3
# speedup vs baseline: 1.5774x; 1.5774x over previous
"""Trainium2 Bass kernel v2 for nn_Mel_Decoder.

Data-parallel over batch: 128 -> 16 per NeuronCore (8 cores).

Design (per core, per block of SB=32 decoder steps, pairs j = b*sbk + il):
  - w1enc (fp16) and enc (fp16, t-major) are SBUF-resident for the whole
    kernel; no per-block reload.
  - attention tanh: DVE fp16 4x tensor_scalar adds (w1enc + s per pair)
    into big x16 tiles; single ACT Tanh per FDJ pairs (FD=FDJ*2*512) with
    fp8e3 output e8.
  - scores: per-pair stationary e8 chunks (fp8 -> 4x fast weight load),
    moving v8 [128,1]; psum [t,pair] transposed layout; exp (no max sub);
    denominator via ones-matmul over t partitions; 1/den folded into d_dot.
  - GRU gates use tanh instead of sigmoid (sigma(x) = (1+tanh(x/2))/2) so
    the whole kernel uses ONE ACT table set (exp_and_others: exp/tanh/relu).
  - GRU states kept fp32 with fp16 mirrors (matmul moving operands).
  - software pipeline: att-GRU chain of block k+1 and GRU1/2 chain of
    block k-1 are emitted interleaved into block k's attention phase.
"""

import os
from contextlib import ExitStack

import numpy as np

import concourse.bass as bass  # noqa: F401
import concourse.mybir as mybir
import concourse.tile as tile
from concourse import bacc
from concourse.bass_utils import run_bass_kernel_spmd
from concourse.masks import make_identity

FP32 = mybir.dt.float32
FP16 = mybir.dt.float16
FP8 = mybir.dt.float8e3
AF = mybir.ActivationFunctionType
ALU = mybir.AluOpType

P = 128
H = 256
H2 = 128
G3 = 768
MEL = 80
R = 5
TENC = 512
TDEC = 1000
BS = 128
NCORE = 8
BL = BS // NCORE
NSTEP = int(os.environ.get("MELDEC_STEPS", TDEC // R))  # 200
SB = 32
FDJ = 4           # pairs per tanh ACT instruction (FD = FDJ*2*512 = 4096)

NPAIR = NSTEP * BL


def _blocks():
    out = []
    s = 0
    while s < NSTEP:
        out.append((s, min(SB, NSTEP - s)))
        s += SB
    return out


def ts(i, n):
    return slice(i * n, (i + 1) * n)


class Builder:
    def __init__(self, nc, tc):
        self.nc = nc
        self.tc = tc

    # --------------------------------------------------------------- helpers
    def load_transposed(self, pool, ps_pool, w_ap, Mdim, Kdim, name):
        """fp32 sbuf tile [128, ceil(K/128), M]; dst[p,kc,m] = w[m, kc*128+p]."""
        nc = self.nc
        KC = (Kdim + P - 1) // P
        dst = pool.tile([P, KC, Mdim], FP32, tag=name)
        if Kdim % P != 0:
            nc.vector.memset(dst[:], 0.0)
        nrc = (Mdim + P - 1) // P
        for rc in range(nrc):
            rcnt = min(P, Mdim - rc * P)
            wrow = pool.tile([P, Kdim], FP32, name="wstg", tag="wstg",
                             padded_shape=[P, G3])
            nc.sync.dma_start(wrow[:rcnt, :], w_ap[rc * P : rc * P + rcnt, :])
            for kc in range(KC):
                kf = min(P, Kdim - kc * P)
                pst = ps_pool.tile([P, P], FP32, name="tr", tag="tr")
                nc.tensor.transpose(
                    pst[:kf, :rcnt], wrow[:rcnt, kc * P : kc * P + kf],
                    self.ident[:rcnt, :rcnt],
                )
                nc.vector.tensor_copy(dst[:kf, kc, rc * P : rc * P + rcnt],
                                      pst[:kf, :rcnt])
        return dst

    def lt16(self, stage_pool, pool, ps_pool, w_ap, Mdim, Kdim, name,
             dbl_n=False):
        """Transposed weight as fp16.  dbl_n: scale cols [2H,3H) by 2
        (pre-doubled n-gate input rows for the gate fusion)."""
        nc = self.nc
        t32 = self.load_transposed(stage_pool, ps_pool, w_ap, Mdim, Kdim,
                                   "w32stg")
        KC = (Kdim + P - 1) // P
        t16 = pool.tile([P, KC, Mdim], FP16, tag=name)
        if dbl_n:
            assert Mdim == G3
            nc.vector.tensor_copy(t16[:, :, 0 : 2 * H], t32[:, :, 0 : 2 * H])
            nc.vector.tensor_scalar_mul(t16[:, :, 2 * H : G3],
                                        t32[:, :, 2 * H : G3], 2.0)
        else:
            nc.vector.tensor_copy(t16[:], t32[:])
        return t16

    def load_vec(self, pool, v_ap, L, name):
        nc = self.nc
        t = pool.tile([P, L // P], FP32, tag=name)
        nc.sync.dma_start(t[:], v_ap.rearrange("(c p) -> p c", p=P))
        return t

    # ---------------------------------------------------------------- setup
    def setup(self, ins, psp):
        nc = self.nc
        cp = self.const

        self.ident = cp.tile([P, P], FP32, name="ident", tag="ident")
        make_identity(nc, self.ident[:])
        self.ident16 = cp.tile([P, P], FP16, name="ident16", tag="ident16")
        nc.vector.tensor_copy(self.ident16[:], self.ident[:])

        with self.tc.tile_pool(name="wsetup", bufs=1) as wsp:
            lt = lambda ap, M, K, nm, dbl=False: self.lt16(
                wsp, cp, psp, ap, M, K, nm, dbl)
            self.att_whhT = lt(ins["att_whh"], G3, H, "att_whhT")
            self.g1_whhT = lt(ins["g1_whh"], G3, H, "g1_whhT")
            self.g2_whhT = lt(ins["g2_whh"], G3, H, "g2_whhT")
            self.g1_wihT = lt(ins["g1_wih"], G3, H, "g1_wihT", dbl=True)
            self.g2_wihT = lt(ins["g2_wih"], G3, H, "g2_wihT", dbl=True)
            self.w2T = lt(ins["w2"], H, H, "w2T")
            self.projT = lt(ins["proj_w"], H, 2 * H, "projT")
            self.outwT = lt(ins["out_w"], MEL * R, H, "outwT")
            self.w1T = lt(ins["w1"], H, H, "w1T")

        self.b1T = self.load_vec(cp, ins["b1"], H, "b1T")
        self.b2T = self.load_vec(cp, ins["b2"], H, "b2T")
        self.proj_bT = self.load_vec(cp, ins["proj_b"], H, "proj_bT")
        self.pre_b1T = self.load_vec(cp, ins["pre_b1"], H, "pre_b1T")
        self.pre_b2T = self.load_vec(cp, ins["pre_b2"], H2, "pre_b2T")

        vf = cp.tile([P, 2], FP32, name="vf", tag="vf")
        nc.sync.dma_start(vf[:], ins["v_w"][0].rearrange("(c p) -> p c", p=P))
        # v scaled x16 into fp8e3's normal range (|v|~0.05 is subnormal
        # territory otherwise); the 1/16 is folded into the Exp scale.
        self.v8 = cp.tile([P, 2], FP8, name="v8", tag="v8")
        nc.vector.tensor_scalar_mul(self.v8[:], vf[:], 16.0)

        self.ones_col16 = cp.tile([P, 1], FP16, name="ones_col16",
                                  tag="ones_col16")
        nc.vector.memset(self.ones_col16[:], 1.0)
        self.ones_row16 = cp.tile([1, P], FP16, name="ones_row16",
                                  tag="ones_row16")
        nc.vector.memset(self.ones_row16[:], 1.0)
        ones_row = cp.tile([1, P], FP32, name="ones_row", tag="ones_row")
        nc.vector.memset(ones_row[:], 1.0)

        self.gi_sb0 = cp.tile([P, 6, BL * SB], FP16, tag="gi_sb0")

        ob_row = cp.tile([1, MEL * R], FP32, name="ob_row", tag="ob_row")
        nc.sync.dma_start(ob_row[:], ins["out_b"][None, :])
        ps_ob = psp.tile([P, MEL * R], FP32, name="mm", tag="mm")
        nc.tensor.matmul(ps_ob[:], ones_row[:], ob_row[:], start=True,
                         stop=True)
        self.outbB = cp.tile([P, MEL * R], FP32, name="outbB", tag="outbB")
        nc.scalar.copy(self.outbB[:], ps_ob[:])

    # -------------------------------------------- enc16 + w1enc16 (SBUF-resident)
    def enc_setup(self, ins):
        nc, tc = self.nc, self.tc
        enc = ins["enc_vec"]
        cp = self.const
        # enc16[p, b, tc4, h]  : t on partitions (for d_dot stationary)
        self.enc16 = cp.tile([P, BL, 4, H], FP16, tag="enc16")
        # w1enc16[p, c, b, t]  : h on partitions (for the bias-add input)
        self.w1enc16 = cp.tile([P, 2, BL, TENC], FP16, tag="w1enc16")
        with tc.tile_pool(name="encs", bufs=3) as ep, \
             tc.tile_pool(name="ps_enc", bufs=2, space="PSUM") as psp:
            for b in range(BL):
                st32 = ep.tile([P, 4, H], FP32, name="enc_stg", tag="enc_stg")
                nc.sync.dma_start(st32[:],
                                  enc[b].rearrange("(t p) h -> p t h", p=P))
                nc.vector.tensor_copy(self.enc16[:, b], st32[:])
                # transpose to [h, t] (fp16) and matmul w1T
                encT = ep.tile([P, 2, TENC], FP16, name="encT", tag="encT")
                for t4 in range(4):
                    for hc in range(2):
                        pst = psp.tile([P, P], FP16, name="tr16", tag="tr16")
                        nc.tensor.transpose(pst[:],
                                            self.enc16[:, b, t4, ts(hc, P)],
                                            self.ident16[:])
                        nc.vector.tensor_copy(encT[:, hc, ts(t4, P)], pst[:])
                for m in range(2):
                    ps = psp.tile([P, TENC], FP32, name="mm", tag="mm")
                    for k in range(2):
                        nc.tensor.matmul(ps[:], self.w1T[:, k, ts(m, P)],
                                         encT[:, k, :],
                                         start=(k == 0), stop=(k == 1))
                    nc.scalar.copy(self.w1enc16[:, m, b], ps[:])

    # ------------------------------------------------- pre-net + gi (DRAM, fp16)
    def prenet(self, ins, gi_d):
        nc, tc = self.nc, self.tc
        dec = ins["decoder_input"]
        with tc.tile_pool(name="pre2", bufs=2) as pp, \
             tc.tile_pool(name="pre1", bufs=1) as pp1, \
             tc.tile_pool(name="ps_pre", bufs=2, space="PSUM") as psp:
            prew1T = self.lt16(pp1, pp1, psp, ins["pre_w1"], H, MEL,
                               "prew1T")
            prew2T = self.lt16(pp1, pp1, psp, ins["pre_w2"], H2, H,
                               "prew2T")
            att_wihT = self.lt16(pp1, pp1, psp, ins["att_wih"], G3, H2,
                                 "att_wihT")

            xsrT = pp1.tile([P, NPAIR], FP16, name="xsrT", tag="xsrT")
            nc.vector.memset(xsrT[:], 0.0)
            for s0, sbk in _blocks():
                gb = P // sbk
                for t0 in range((BL * sbk) // P):
                    xt = pp.tile([P, MEL], FP32, name="xsr_nat", tag="xsr_nat")
                    src = dec[t0 * gb : (t0 + 1) * gb,
                              s0 * R : (s0 + sbk) * R : R, :]
                    nc.sync.dma_start(xt[:], src)
                    pst = psp.tile([P, P], FP32, name="tr", tag="tr")
                    nc.tensor.transpose(pst[:MEL, :], xt[:, :], self.ident[:])
                    nc.scalar.copy(
                        xsrT[:MEL, BL * s0 + t0 * P : BL * s0 + (t0 + 1) * P],
                        pst[:MEL, :])

            # chunk-outer so block 0's gi lands early (A(0) overlaps the rest)
            for n0 in range(0, NPAIR, 512):
                nsz = min(512, NPAIR - n0)
                pre1T = pp.tile([P, 2, 512], FP16, name="pre1T", tag="pre1T")
                xsT = pp.tile([P, 512], FP16, name="xsT", tag="xsT")
                for m in range(2):
                    ps = psp.tile([P, 512], FP32, name="mm", tag="mm")
                    nc.tensor.matmul(ps[:, :nsz], prew1T[:, 0, ts(m, P)],
                                     xsrT[:, n0 : n0 + nsz],
                                     start=True, stop=True)
                    nc.vector.tensor_scalar(pre1T[:, m, :nsz], ps[:, :nsz],
                                            self.pre_b1T[:, m : m + 1], 0.0,
                                            ALU.add, ALU.max)
                ps = psp.tile([P, 512], FP32, name="mm", tag="mm")
                for k in range(2):
                    nc.tensor.matmul(ps[:, :nsz], prew2T[:, k, :],
                                     pre1T[:, k, :nsz],
                                     start=(k == 0), stop=(k == 1))
                nc.vector.tensor_scalar(xsT[:, :nsz], ps[:, :nsz],
                                        self.pre_b2T[:, 0:1], 0.0,
                                        ALU.add, ALU.max)
                for m in range(6):
                    ps = psp.tile([P, 512], FP32, name="mm", tag="mm")
                    nc.tensor.matmul(ps[:, :nsz], att_wihT[:, 0, ts(m, P)],
                                     xsT[:, :nsz], start=True, stop=True)
                    # block 0's gi goes straight to SBUF (no DRAM roundtrip,
                    # whose whole-tensor dep would stall A(0) on all of prenet)
                    dst = (self.gi_sb0[:, m, :nsz] if n0 == 0
                           else pp.tile([P, 512], FP16, name="gi_stage",
                                        tag="gi_stage")[:, :nsz])
                    if m >= 4:
                        # n-gate input pre-doubled for the gate fusion
                        nc.vector.tensor_scalar(dst, ps[:, :nsz],
                                                0.0, 2.0, ALU.add, ALU.mult)
                    else:
                        nc.vector.tensor_copy(dst, ps[:, :nsz])
                    if n0 != 0:
                        nc.sync.dma_start(gi_d[m, :, n0 : n0 + nsz], dst)

    # ---------------------------------------------------------------- gates
    def gates(self, rz_ps, hn_ps, gin, prev16, prev32, out32, out16):
        """GRU cell tail.  rz_ps: psum [128,4,BL] pre-activations for (r,z)
        (both H-chunks); hn_ps: psum [128,2,BL] recurrent n term; gin:
        PRE-DOUBLED n input term (sbuf fp16 or psum fp32).
        sigma(x) = (1+tanh(x/2))/2 so only Tanh is used.
        h' = 0.5*(h + n + tz*(h-n)), n = tanh(0.5*(2*gin + hn + tr*hn))."""
        nc = self.nc
        g = self.g_pool
        trz = g.tile([P, 4, BL], FP16, name="trz", tag="trz")
        nc.scalar.activation(trz[:], rz_ps, AF.Tanh, scale=0.5)
        s1 = g.tile([P, 2, BL], FP32, name="s1", tag="s1")
        nc.vector.tensor_mul(s1[:], trz[:, 0:2], hn_ps)
        s2 = g.tile([P, 2, BL], FP32, name="s2", tag="s2")
        nc.vector.tensor_add(s2[:], s1[:], hn_ps)
        s3 = g.tile([P, 2, BL], FP32, name="s3", tag="s3")
        nc.vector.tensor_add(s3[:], s2[:], gin)
        tn = g.tile([P, 2, BL], FP16, name="tn", tag="tn")
        nc.scalar.activation(tn[:], s3[:], AF.Tanh, scale=0.5)
        a = g.tile([P, 2, BL], FP16, name="ga", tag="ga")
        nc.vector.tensor_sub(a[:], prev16, tn[:])
        bq = g.tile([P, 2, BL], FP16, name="gb", tag="gb")
        nc.vector.tensor_mul(bq[:], trz[:, 2:4], a[:])
        q = g.tile([P, 2, BL], FP16, name="gq", tag="gq")
        nc.vector.tensor_add(q[:], bq[:], tn[:])
        t32 = g.tile([P, 2, BL], FP32, name="gt", tag="gt")
        nc.vector.tensor_add(t32[:], prev32, q[:])
        nc.vector.tensor_scalar_mul(out32, t32[:], 0.5)
        nc.vector.tensor_scalar_mul(out16, t32[:], 0.5)

    # ------------------------------------------------------------ chain steps
    def emit_A_step(self, il, sbk, bp, gi_blk, dT16, sT16):
        nc = self.nc
        sl = slice(il, bp, sbk)
        ps = self.ps_gruA.tile([P, 8, BL], FP32, name="gruA", tag="gruA")
        for m in range(4):
            nc.tensor.matmul(ps[:, m], self.att_whhT[:, 0, ts(m, P)],
                             self.d16_prev, start=True, stop=False)
            nc.tensor.matmul(ps[:, m], self.att_whhT[:, 1, ts(m, P)],
                             self.d16_prev2, start=False, stop=False)
            nc.tensor.matmul(ps[:, m], self.ident16[:], gi_blk[:, m, sl],
                             start=False, stop=True)
        for m in (4, 5):
            nc.tensor.matmul(ps[:, m], self.att_whhT[:, 0, ts(m, P)],
                             self.d16_prev, start=True, stop=False)
            nc.tensor.matmul(ps[:, m], self.att_whhT[:, 1, ts(m, P)],
                             self.d16_prev2, start=False, stop=True)
        d32 = self.st_pool.tile([P, 2, BL], FP32, name="dA32", tag="dA32")
        self.gates(ps[:, 0:4], ps[:, 4:6], gi_blk[:, 4:6, sl],
                   self.dfull16_prev, self.dfull32_prev,
                   d32[:], dT16[:, :, sl])
        self.d16_prev = dT16[:, 0, sl]
        self.d16_prev2 = dT16[:, 1, sl]
        self.dfull16_prev = dT16[:, :, sl]
        self.dfull32_prev = d32[:]
        # per-step sT = w2 @ d + b2 (lets the next block's attention start
        # before this chain is fully done)
        for m in range(2):
            for k in range(2):
                nc.tensor.matmul(ps[:, 6 + m], self.w2T[:, k, ts(m, P)],
                                 dT16[:, k, sl], start=(k == 0), stop=(k == 1))
            nc.vector.tensor_scalar_add(sT16[:, m, sl], ps[:, 6 + m],
                                        self.b2T[:, m : m + 1])

    def emit_G_step(self, il, sbk, bp, pT16, G1P16, sum2T16):
        nc = self.nc
        sl = slice(il, bp, sbk)
        # GRU1
        ps1 = self.ps_gru1.tile([P, 6, BL], FP32, name="gru1", tag="gru1")
        for m in range(4):
            nc.tensor.matmul(ps1[:, m], self.g1_whhT[:, 0, ts(m, P)],
                             self.o1_16[:, 0], start=True, stop=False)
            nc.tensor.matmul(ps1[:, m], self.g1_whhT[:, 1, ts(m, P)],
                             self.o1_16[:, 1], start=False, stop=False)
            nc.tensor.matmul(ps1[:, m], self.ident16[:], G1P16[:, m, sl],
                             start=False, stop=True)
        for m in (4, 5):
            nc.tensor.matmul(ps1[:, m], self.g1_whhT[:, 0, ts(m, P)],
                             self.o1_16[:, 0], start=True, stop=False)
            nc.tensor.matmul(ps1[:, m], self.g1_whhT[:, 1, ts(m, P)],
                             self.o1_16[:, 1], start=False, stop=True)
        st = self.st_pool
        o1n32 = st.tile([P, 2, BL], FP32, name="o1n32", tag="o1n32")
        o1n16 = st.tile([P, 2, BL], FP16, name="o1n16", tag="o1n16")
        self.gates(ps1[:, 0:4], ps1[:, 4:6], G1P16[:, 4:6, sl],
                   self.o1_16[:], self.o1_32[:], o1n32[:], o1n16[:])
        in2_32 = st.tile([P, 2, BL], FP32, name="in2_32", tag="in2_32")
        in2_16 = st.tile([P, 2, BL], FP16, name="in2_16", tag="in2_16")
        nc.vector.tensor_add(in2_32[:], o1n32[:], pT16[:, :, sl])
        nc.vector.tensor_copy(in2_16[:], in2_32[:])
        # GRU2: rz 0:4 (whh+wih), hn 4:6 (whh n), inn 6:8 (wih n, pre-doubled)
        ps2 = self.ps_gru2.tile([P, 8, BL], FP32, name="gru2", tag="gru2")
        for m in range(4):
            nc.tensor.matmul(ps2[:, m], self.g2_whhT[:, 0, ts(m, P)],
                             self.o2_16[:, 0], start=True, stop=False)
            nc.tensor.matmul(ps2[:, m], self.g2_whhT[:, 1, ts(m, P)],
                             self.o2_16[:, 1], start=False, stop=False)
            nc.tensor.matmul(ps2[:, m], self.g2_wihT[:, 0, ts(m, P)],
                             in2_16[:, 0], start=False, stop=False)
            nc.tensor.matmul(ps2[:, m], self.g2_wihT[:, 1, ts(m, P)],
                             in2_16[:, 1], start=False, stop=True)
        for m in range(2):
            nc.tensor.matmul(ps2[:, 4 + m], self.g2_whhT[:, 0, ts(4 + m, P)],
                             self.o2_16[:, 0], start=True, stop=False)
            nc.tensor.matmul(ps2[:, 4 + m], self.g2_whhT[:, 1, ts(4 + m, P)],
                             self.o2_16[:, 1], start=False, stop=True)
            nc.tensor.matmul(ps2[:, 6 + m], self.g2_wihT[:, 0, ts(4 + m, P)],
                             in2_16[:, 0], start=True, stop=False)
            nc.tensor.matmul(ps2[:, 6 + m], self.g2_wihT[:, 1, ts(4 + m, P)],
                             in2_16[:, 1], start=False, stop=True)
        o2n32 = st.tile([P, 2, BL], FP32, name="o2n32", tag="o2n32")
        o2n16 = st.tile([P, 2, BL], FP16, name="o2n16", tag="o2n16")
        self.gates(ps2[:, 0:4], ps2[:, 4:6], ps2[:, 6:8],
                   self.o2_16[:], self.o2_32[:], o2n32[:], o2n16[:])
        nc.vector.tensor_add(sum2T16[:, :, sl], in2_16[:], o2n16[:])
        self.o1_32, self.o1_16 = o1n32, o1n16
        self.o2_32, self.o2_16 = o2n32, o2n16

    # ---------------------------------------------------------------- phases
    def emit_T(self, sbk, bp, sT16, a_tokens, g_tokens, il_major=False):
        nc = self.nc
        ng = bp // P
        gb = P // sbk
        nchunk = bp // FDJ
        # A tokens (and the trailing S) run at every chunk from the start so
        # the next block's sT is ready early; G tokens spread over the rest.
        na = len(a_tokens)
        g_start = na + 1
        g_every = max(1, (nchunk - g_start) // max(1, len(g_tokens)))
        tokens = {}
        for i, t in enumerate(a_tokens):
            tokens[i] = t
        for i, t in enumerate(g_tokens):
            tokens[g_start + i * g_every] = t
        ti = 0
        expT = self.expT_pool.tile([P, 4, BL * SB], FP16, name="expT",
                                   tag="expT")[:, :, :bp]
        rdenB = self.expT_pool.tile([P, BL * SB], FP16, name="rdenB",
                                    tag="rdenB")[:, :bp]
        ps_dd = self.ps_dd.tile([P, 2, 512], FP32, name="dd",
                                tag="dd")[:, :, :bp]
        ps_s = None
        for g in range(ng):
            if il_major:
                starts = [b * sbk + il4 * FDJ
                          for il4 in range(sbk // FDJ)
                          for b in range(g * gb, (g + 1) * gb)]
            else:
                starts = [g * P + q * FDJ for q in range(P // FDJ)]
            for jq, j0 in enumerate(starts):
                x16 = self.x_pool.tile([P, FDJ, 2, TENC], FP16, name="x16",
                                       tag="x16")
                for jj in range(FDJ):
                    j = j0 + jj
                    b = j // sbk
                    for c in range(2):
                        nc.vector.tensor_scalar_add(
                            x16[:, jj, c], self.w1enc16[:, c, b],
                            sT16[:, c, j : j + 1])
                e8 = self.e_pool.tile([P, FDJ, 2, TENC], FP8, name="e8",
                                      tag="e8")
                nc.scalar.activation(e8[:], x16[:], AF.Tanh)
                if jq == 0:
                    ps_s = self.ps_sc.tile([P, 4, P], FP32, name="sc",
                                           tag="sc")
                for jj in range(FDJ):
                    j = j0 + jj
                    row = j % P
                    for t4 in range(4):
                        for c in range(2):
                            nc.tensor.matmul(
                                ps_s[:, t4, row : row + 1],
                                e8[:, jj, c, ts(t4, P)],
                                self.v8[:, c : c + 1],
                                start=(c == 0), stop=(c == 1))
                tok = tokens.pop(g * (P // FDJ) + jq, None)
                if tok is not None:
                    tok()
            gsl = ts(g, P)
            nc.scalar.activation(expT[:, :, gsl], ps_s[:], AF.Exp,
                                 scale=1.0 / 16.0)
            ps_den = self.ps_mm.tile([1, P], FP32, name="den", tag="mm")
            for t4 in range(4):
                nc.tensor.matmul(ps_den[0:1, :], self.ones_col16[:],
                                 expT[:, t4, gsl],
                                 start=(t4 == 0), stop=(t4 == 3))
            rden16 = self.g_pool.tile([1, P], FP16, name="rden", tag="rden")
            with nc.allow_low_precision(reason="softmax 1/den in fp16"):
                nc.vector.reciprocal(rden16[0:1, :], ps_den[0:1, :])
            ps_rb = self.ps_mm.tile([P, P], FP32, name="rb", tag="mm")
            nc.tensor.matmul(ps_rb[:], self.ones_row16[0:1, :],
                             rden16[0:1, :], start=True, stop=True)
            nc.vector.tensor_copy(rdenB[:, gsl], ps_rb[:])
            for bi in range(gb):
                b = g * gb + bi
                bs_ = slice(b * sbk, (b + 1) * sbk)
                for hc in range(2):
                    for k4 in range(4):
                        nc.tensor.matmul(ps_dd[:, hc, bs_],
                                         self.enc16[:, b, k4, ts(hc, P)],
                                         expT[:, k4, bs_],
                                         start=(k4 == 0), stop=(k4 == 3))
        for idx in sorted(tokens):
            tokens[idx]()
        tokens.clear()
        ddT16 = self.ddT_pool.tile([P, 2, BL * SB], FP16, name="ddT",
                                   tag="ddT")[:, :, :bp]
        for hc in range(2):
            nc.vector.tensor_mul(ddT16[:, hc], ps_dd[:, hc], rdenB[:])
        return ddT16

    def emit_P5(self, dT16, ddT16, bp):
        nc = self.nc
        pT16 = self.pT_pool.tile([P, 2, BL * SB], FP16, name="pT",
                                 tag="pT")[:, :, :bp]
        for m in range(2):
            ps = self.ps_mm.tile([P, 512], FP32, name="mm", tag="mm")[:, :bp]
            for k in range(4):
                rhs = dT16[:, k, :] if k < 2 else ddT16[:, k - 2, :]
                nc.tensor.matmul(ps[:], self.projT[:, k, ts(m, P)], rhs,
                                 start=(k == 0), stop=(k == 3))
            nc.vector.tensor_scalar_add(pT16[:, m], ps[:],
                                        self.proj_bT[:, m : m + 1])
        G1P16 = self.G1P_pool.tile([P, 6, BL * SB], FP16, name="G1P",
                                   tag="G1P")[:, :, :bp]
        for m in range(6):
            ps = self.ps_mm.tile([P, 512], FP32, name="mm", tag="mm")[:, :bp]
            for k in range(2):
                nc.tensor.matmul(ps[:], self.g1_wihT[:, k, ts(m, P)],
                                 pT16[:, k, :], start=(k == 0), stop=(k == 1))
            nc.vector.tensor_copy(G1P16[:, m], ps[:])
        return pT16, G1P16

    def emit_O(self, sum2T16, s0, sbk, bp, y):
        nc = self.nc
        ng = bp // P
        gb = P // sbk
        for t0 in range(ng):
            ps = self.ps_mm.tile([P, MEL * R], FP32, name="mm", tag="mm")
            for k in range(2):
                nc.tensor.matmul(ps[:], sum2T16[:, k, ts(t0, P)],
                                 self.outwT[:, k, :],
                                 start=(k == 0), stop=(k == 1))
            ot = self.o_pool.tile([P, MEL * R], FP32, name="out_sb",
                                  tag="out_sb")
            nc.vector.tensor_add(ot[:], ps[:], self.outbB[:])
            for bi in range(gb):
                b = t0 * gb + bi
                nc.sync.dma_start(
                    y[b, s0 * R : (s0 + sbk) * R, :].rearrange(
                        "(i r) m -> i (r m)", r=R),
                    ot[bi * sbk : (bi + 1) * sbk, :])

    # ------------------------------------------------------------------ main
    def main(self, ins, gi_d, y, stack):
        nc, tc = self.nc, self.tc
        ec = stack.enter_context
        self.st_pool = ec(tc.tile_pool(name="states", bufs=2))
        self.g_pool = ec(tc.tile_pool(name="gates", bufs=3))
        self.dT_pool = ec(tc.tile_pool(name="dT", bufs=3))
        self.sT_pool = ec(tc.tile_pool(name="sT", bufs=2))
        self.pT_pool = ec(tc.tile_pool(name="pT", bufs=2))
        self.G1P_pool = ec(tc.tile_pool(name="G1P", bufs=2))
        self.sum2_pool = ec(tc.tile_pool(name="sum2", bufs=2))
        self.ddT_pool = ec(tc.tile_pool(name="ddT", bufs=2))
        self.expT_pool = ec(tc.tile_pool(name="expT", bufs=2))
        self.x_pool = ec(tc.tile_pool(name="xbuf", bufs=2))
        self.e_pool = ec(tc.tile_pool(name="ebuf", bufs=2))
        self.gi_pool = ec(tc.tile_pool(name="gi", bufs=2))
        self.o_pool = ec(tc.tile_pool(name="obuf", bufs=2))
        self.ps_sc = ec(tc.tile_pool(name="ps_sc", bufs=1, space="PSUM"))
        self.ps_dd = ec(tc.tile_pool(name="ps_dd", bufs=1, space="PSUM"))
        self.ps_mm = ec(tc.tile_pool(name="ps_mm", bufs=1, space="PSUM"))
        self.ps_gruA = ec(tc.tile_pool(name="ps_gruA", bufs=2, space="PSUM"))
        self.ps_gru1 = ec(tc.tile_pool(name="ps_gru1", bufs=1, space="PSUM"))
        self.ps_gru2 = ec(tc.tile_pool(name="ps_gru2", bufs=1, space="PSUM"))

        zeros16 = self.st_pool.tile([P, 2, BL], FP16, name="z16", tag="z16")
        nc.vector.memset(zeros16[:], 0.0)
        zeros32 = self.st_pool.tile([P, 2, BL], FP32, name="z32", tag="z32")
        nc.vector.memset(zeros32[:], 0.0)
        self.d16_prev = zeros16[:, 0]
        self.d16_prev2 = zeros16[:, 1]
        self.dfull16_prev = zeros16[:]
        self.dfull32_prev = zeros32[:]
        self.o1_16 = zeros16
        self.o1_32 = zeros32
        self.o2_16 = zeros16
        self.o2_32 = zeros32

        blocks = _blocks()

        def load_gi(k):
            s0, sbk = blocks[k]
            bp = BL * sbk
            p0 = BL * s0
            gi_blk = self.gi_pool.tile([P, 6, BL * SB], FP16, name="gi_blk",
                                       tag="gi_blk")[:, :, :bp]
            nc.sync.dma_start(
                gi_blk[:],
                gi_d[:, :, p0 : p0 + bp].rearrange("c p j -> p c j"))
            return gi_blk

        def new_dT(bp_):
            dT = self.dT_pool.tile([P, 2, BL * SB], FP16, name="dT16",
                                   tag="dT16")[:, :, :bp_]
            sT = self.sT_pool.tile([P, 2, BL * SB], FP32, name="sT",
                                   tag="sT")[:, :, :bp_]
            return dT, sT

        # prologue: A(0) chain (computes sT(0) per step)
        gi_blk = self.gi_sb0
        s0, sbk = blocks[0]
        bp = BL * sbk
        dT16, sT16 = new_dT(bp)
        for il in range(sbk):
            self.emit_A_step(il, sbk, bp, gi_blk, dT16, sT16)

        dT16_cur = dT16
        sum2_prev = None
        prev_blk = None       # (s0, sbk, bp) of block k-1
        pT_prev = G1P_prev = None

        for k in range(len(blocks)):
            s0, sbk = blocks[k]
            bp = BL * sbk
            a_tokens = []
            g_tokens = []
            # A(k+1) tokens (+ S(k+1) right after the chain completes)
            if k + 1 < len(blocks):
                s0n, sbkn = blocks[k + 1]
                bpn = BL * sbkn
                gi_next = load_gi(k + 1)
                dT16n, sT16n = new_dT(bpn)
                for il in range(sbkn):
                    a_tokens.append(lambda il=il: self.emit_A_step(
                        il, sbkn, bpn, gi_next, dT16n, sT16n))
            # G(k-1) tokens
            if prev_blk is not None:
                ps0, psbk, pbp = prev_blk
                sum2_prev = self.sum2_pool.tile(
                    [P, 2, BL * SB], FP16, name="sum2", tag="sum2")[:, :, :pbp]
                for il in range(psbk):
                    g_tokens.append(lambda il=il, s=sum2_prev: self.emit_G_step(
                        il, psbk, pbp, pT_prev, G1P_prev, s))

            ddT16 = self.emit_T(sbk, bp, sT16, a_tokens, g_tokens,
                                il_major=(k == 0))
            if k + 1 < len(blocks):
                sT16 = sT16n
            pT_prev, G1P_prev = self.emit_P5(dT16_cur, ddT16, bp)
            if prev_blk is not None:
                self.emit_O(sum2_prev, prev_blk[0], prev_blk[1], prev_blk[2], y)
            prev_blk = (s0, sbk, bp)
            if k + 1 < len(blocks):
                dT16_cur = dT16n

        # epilogue: G(last) + O(last)
        ps0, psbk, pbp = prev_blk
        sum2_last = self.sum2_pool.tile([P, 2, BL * SB], FP16, name="sum2",
                                        tag="sum2")[:, :, :pbp]
        for il in range(psbk):
            self.emit_G_step(il, psbk, pbp, pT_prev, G1P_prev, sum2_last)
        self.emit_O(sum2_last, ps0, psbk, pbp, y)


def build(ins_np):
    for nm in ("att_bih", "att_bhh", "g1_bih", "g1_bhh", "g2_bih", "g2_bhh",
               "b1"):
        assert not np.asarray(ins_np[nm]).any(), f"nonzero gate bias {nm}"
    nc = bacc.Bacc()
    ins = {}
    for name, arr in ins_np.items():
        shp = list(np.asarray(arr).shape)
        if name in ("enc_vec", "decoder_input"):
            shp[0] = BL
        ins[name] = nc.declare_dram_parameter(name, shp, FP32, isOutput=False)
    y = nc.declare_dram_parameter("y", [BL, TDEC, MEL], FP32, isOutput=True)
    gi_d = nc.dram_tensor("gi_d", [6, P, NPAIR], FP16)

    with tile.TileContext(nc) as tc:
        with ExitStack() as stack:
            b = Builder(nc, tc)
            b.const = stack.enter_context(tc.tile_pool(name="const", bufs=1))
            with tc.tile_pool(name="ps_setup", bufs=2, space="PSUM") as psp:
                b.setup(ins, psp)
            b.prenet(ins, gi_d)
            b.enc_setup(ins)
            b.main(ins, gi_d, y, stack)
    nc.compile()
    return nc


_CACHE = {}


def kernel(**inputs):
    if "nc" not in _CACHE:
        _CACHE["nc"] = build(inputs)
    nc = _CACHE["nc"]
    in_maps = []
    for c in range(NCORE):
        m = {}
        for name, arr in inputs.items():
            a = np.asarray(arr, dtype=np.float32)
            if name in ("enc_vec", "decoder_input"):
                a = a[c * BL : (c + 1) * BL]
            m[name] = np.ascontiguousarray(a)
        in_maps.append(m)
    res = run_bass_kernel_spmd(nc, in_maps, list(range(NCORE)))
    return np.concatenate([res.results[c]["y"] for c in range(NCORE)], axis=0)
